# revision 21
# baseline (speedup 1.0000x reference)
"""Trainium2 Bass kernel for DeepRadAEVComputer (B=8, N=256).

Sharding: data-parallel over batch dim B — each of the 8 NeuronCores
processes one molecule (d [256,256], z [256]); weights replicated.

Per-core algorithm (feature-major MLP):
  mask/fc/chem features computed point-major at full 128-partition width,
  then shuffled into feature-major rhs blocks ([3,512] rows per 512-point
  group packed 4 groups per [128,512] SBUF tile at partition bases
  0/32/64/96). The 7-layer tanh MLP runs feature-major with two K=64
  matmuls packed on the PE via disjoint (row,col) tile positions, so the
  per-layer psum is partition-stacked [128, 2048] and every tanh runs at
  full 128-partition ACT width. The cutoff-weighted sum over neighbors j
  is a PE-ones partition-broadcast of w followed by a segmented DVE
  reduce; the final row-normalize uses a PE ones-matvec for the
  partition sum, and the [feat, i] result is PE-transposed for a
  contiguous store. ACT Sin is inaccurate on TRN2, so the cutoff cosine
  is evaluated as an even minimax polynomial on the DVE.
"""

import math
import sys

import numpy as np

sys.path.insert(0, "/opt/trn_rl_repo")

import concourse.bass as bass  # noqa: E402
import concourse.tile as tile  # noqa: E402
from concourse import bacc, masks, mybir  # noqa: E402
from concourse.bass_utils import run_bass_kernel_spmd  # noqa: E402

F32 = mybir.dt.float32
BF16 = mybir.dt.bfloat16
I32 = mybir.dt.int32
AF = mybir.ActivationFunctionType
OP = mybir.AluOpType

N = 256          # atoms per molecule
NPTS = N * N     # 65536 pair-points per core
GSZ = 512        # points per group (2 d-rows)
NGRP = NPTS // GSZ          # 128 groups
SUPER = 8                   # groups per super-tile
NSUP = NGRP // SUPER        # 16 super-tiles
CUTOFF = 5.2
EPS = 1e-7
PI = math.pi

# cos(u)-1 = sum_{k>=1} g_k (u^2)^k minimax on [-pi, pi] (max err 4.6e-10)
GCOS = [
    -0.49999999920407, 0.041666664531272414, -0.0013888869362829136,
    2.4800740359237187e-05, -2.7537667614463017e-07, 2.0625484749459024e-09,
    -9.788307045326216e-12,
]

_BUILT = None


def _build():
    nc = bacc.Bacc("TRN2", target_bir_lowering=False, debug=False)

    d_in = nc.dram_tensor("d", [N, N], F32, kind="ExternalInput")
    z_in = nc.dram_tensor("z", [N], I32, kind="ExternalInput")
    w_dram = []
    b_dram = []
    dims = [(3, 64), (64, 64), (64, 64), (64, 64), (64, 64), (64, 64), (64, 128)]
    for li, (fi, fo) in enumerate(dims):
        w_dram.append(nc.dram_tensor(f"W{li}", [fi, fo], F32, kind="ExternalInput"))
        b_dram.append(nc.dram_tensor(f"b{li}", [fo], F32, kind="ExternalInput"))
    out_dram = nc.dram_tensor("out", [N, 128], F32, kind="ExternalOutput")

    d_flat = d_in[:].rearrange("a b -> (a b)")

    with tile.TileContext(nc) as tc:
        with (
            tc.tile_pool(name="const", bufs=1) as cpool,
            tc.tile_pool(name="prep", bufs=1) as ppool,
            tc.tile_pool(name="scratch", bufs=2) as spool,
            tc.tile_pool(name="rhs", bufs=3) as rpool,
            tc.tile_pool(name="wb", bufs=4) as wpool,
            tc.tile_pool(name="hbuf", bufs=12) as hpool,
            tc.tile_pool(name="psum", bufs=2, space="PSUM") as qpool,
        ):
            # ---- constants ----
            ones_t = cpool.tile([128, 128], F32)
            nc.gpsimd.memset(ones_t[:], 1.0)
            ident = cpool.tile([128, 128], F32)
            masks.make_identity(nc, ident[:])
            eps12 = cpool.tile([128, 1], F32)
            nc.gpsimd.memset(eps12[:], 1e-12)

            # weights: stacked copies so both partition-halves of the PE can
            # run the same layer on two point-groups via tile positions.
            # DMA'd in fp32, then converted to bf16 (PE runs 1 cycle/row on
            # bf16 vs 4 on fp32).
            wt = []
            for li, (fi, fo) in enumerate(dims):
                if li == 0:
                    tf = cpool.tile([128, 64], F32, tag="W0f")
                    for k in range(4):
                        nc.sync.dma_start(tf[32 * k : 32 * k + 3, :], w_dram[0][:])
                    t = cpool.tile([128, 64], BF16, tag="W0")
                    for k in range(4):
                        nc.vector.tensor_copy(
                            t[32 * k : 32 * k + 3, :], tf[32 * k : 32 * k + 3, :]
                        )
                else:
                    tf = cpool.tile([128, fo], F32, tag=f"W{li}f")
                    nc.sync.dma_start(tf[0:64, :], w_dram[li][:])
                    nc.sync.dma_start(tf[64:128, :], w_dram[li][:])
                    t = cpool.tile([128, fo], BF16, tag=f"W{li}")
                    nc.vector.tensor_copy(t[:], tf[:])
                wt.append(t)
            bt = []
            for li, (fi, fo) in enumerate(dims):
                t = cpool.tile([128, 1], F32, tag=f"b{li}")
                bcol = b_dram[li][:].rearrange("(p f) -> p f", f=1)
                if fo == 64:
                    nc.sync.dma_start(t[0:64, :], bcol)
                    nc.sync.dma_start(t[64:128, :], bcol)
                else:
                    nc.sync.dma_start(t[:], bcol)
                bt.append(t)

            # ---- prep: chem + cutoff weights, point-major [i-part, j-free] ----
            zrow_i = ppool.tile([1, N], I32)
            nc.sync.dma_start(zrow_i[:], z_in[:].rearrange("(p f) -> p f", p=1))
            zrow = ppool.tile([1, N], F32)
            nc.vector.tensor_copy(zrow[:], zrow_i[:])

            ps_z = qpool.tile([128, 2048], F32, tag="ps")
            # broadcast z along partitions: ones[1,128].T @ zrow[1,256]
            nc.tensor.matmul(ps_z[:, 0:N], ones_t[0:1, :], zrow[:])

            c1h = []
            c2h = []
            wh = []
            dbh = []
            for hf in range(2):  # i-halves
                zcol_i = ppool.tile([128, 1], I32, tag=f"zcol_i{hf}")
                nc.sync.dma_start(
                    zcol_i[:],
                    z_in[128 * hf : 128 * hf + 128].rearrange("(p f) -> p f", f=1),
                )
                zcol = ppool.tile([128, 1], F32, tag=f"zcol{hf}")
                nc.vector.tensor_copy(zcol[:], zcol_i[:])

                dh = spool.tile([128, N], F32)
                nc.sync.dma_start(dh[:], d_in[128 * hf : 128 * hf + 128, :])
                db = ppool.tile([128, N], BF16, tag=f"db_{hf}")
                nc.scalar.copy(db[:], dh[:])
                dbh.append(db)

                s = spool.tile([128, N], F32)
                nc.vector.tensor_scalar_add(s[:], ps_z[:, 0:N], zcol[:])
                m = spool.tile([128, N], F32)
                nc.vector.tensor_scalar_mul(m[:], ps_z[:, 0:N], zcol[:])
                q = spool.tile([128, N], F32)
                nc.vector.tensor_tensor(q[:], s[:], s[:], OP.mult)
                m2 = spool.tile([128, N], F32)
                nc.vector.tensor_tensor(m2[:], m[:], m[:], OP.mult)
                nc.vector.tensor_tensor(q[:], q[:], m2[:], OP.add)
                sq = spool.tile([128, N], F32)
                nc.scalar.activation(sq[:], q[:], AF.Sqrt, bias=eps12[:])
                fac = spool.tile([128, N], F32)
                nc.vector.reciprocal(fac[:], sq[:])
                c1 = ppool.tile([128, N], BF16, tag=f"c1_{hf}")
                nc.vector.tensor_tensor(c1[:], s[:], fac[:], OP.mult)
                c2 = ppool.tile([128, N], BF16, tag=f"c2_{hf}")
                nc.vector.tensor_tensor(c2[:], m[:], fac[:], OP.mult)

                # w = mask * (0.5*cos(pi*d/CUTOFF)+0.5) via DVE polynomial
                th = spool.tile([128, N], F32)
                nc.vector.tensor_single_scalar(th[:], dh[:], PI / CUTOFF, OP.mult)
                uu = spool.tile([128, N], F32)
                nc.vector.tensor_single_scalar(uu[:], th[:], PI, OP.subtract)
                vv = spool.tile([128, N], F32)
                nc.vector.tensor_tensor(vv[:], uu[:], uu[:], OP.mult)
                pa = spool.tile([128, N], F32)
                pb = spool.tile([128, N], F32)
                nc.vector.tensor_single_scalar(pa[:], vv[:], GCOS[6], OP.mult)
                cur, nxt = pa, pb
                for k in range(5, -1, -1):
                    nc.vector.scalar_tensor_tensor(
                        nxt[:], cur[:], GCOS[k], vv[:], OP.add, OP.mult
                    )
                    cur, nxt = nxt, cur
                ne = spool.tile([128, N], F32)
                nc.vector.tensor_single_scalar(ne[:], dh[:], 0.0, OP.not_equal)
                msk = spool.tile([128, N], F32)
                nc.vector.scalar_tensor_tensor(
                    msk[:], dh[:], CUTOFF, ne[:], OP.is_lt, OP.mult
                )
                fcb = spool.tile([128, N], F32)
                nc.vector.tensor_single_scalar(fcb[:], cur[:], -0.5, OP.mult)
                wv = ppool.tile([128, N], BF16, tag=f"w_{hf}")
                nc.vector.tensor_tensor(wv[:], fcb[:], msk[:], OP.mult)
                c1h.append(c1)
                c2h.append(c2)
                wh.append(wv)

            # GR^T accumulator [feat, i]
            grt = ppool.tile([128, N], F32, tag="grt")

            # ---- MLP super-tiles (8 groups = 4096 points each),
            # software-pipelined: the next super's rhs blocks + L0 matmuls
            # are emitted between this super's mid layers so the PE has
            # independent work while each layer's tanh drains (keeps the
            # PE dense and the HAM clock warm) ----
            def build_blocks(sidx):
                # L0 rhs blocks: 4 groups per [128,512] tile,
                # feature rows at partitions 32k+{0,1,2}
                blks = []
                wblks = []
                for bi in range(2):
                    blk = rpool.tile([128, GSZ], BF16, tag="rhs0")
                    c = 2 * sidx + bi  # block index over 32
                    view = blk[:].rearrange("(k r) f -> k r f", r=32)
                    # block c covers groups 4c..4c+3 => i-rows 8c..8c+7
                    hf = (8 * c) // 128
                    r0 = (8 * c) % 128
                    for feat, src in ((0, dbh[hf]), (1, c1h[hf]), (2, c2h[hf])):
                        nc.gpsimd.dma_start(
                            view[:, feat : feat + 1, :], src[r0 : r0 + 8, :]
                        )
                    blks.append(blk)

                    # w broadcast tile: row 0 = this half-super's 8 i-rows of
                    # w, then log2 partition-doubling copies (DMA engines are
                    # nearly idle; saves a K=1 fp32 matmul per group)
                    wb = wpool.tile([128, 2048], BF16, tag="wb")
                    nc.sync.dma_start(wb[0:1, :], wh[hf][r0 : r0 + 8, :])
                    for k in (1, 2, 4, 8, 16, 32, 64):
                        nc.sync.dma_start(wb[k : 2 * k, :], wb[0:k, :])
                    wblks.append(wb)
                return blks, wblks

            if True:
                def mm_layer(ps, rhs_t, w_tile):
                    # 8 matmuls: group j in super; pair p=j//2, half q=j%2
                    for j in range(SUPER):
                        p, qh = j // 2, j % 2
                        nc.tensor.matmul(
                            ps[64 * qh : 64 * qh + 64, GSZ * p : GSZ * (p + 1)],
                            w_tile[64 * qh : 64 * qh + 64, :],
                            rhs_t[
                                64 * qh : 64 * qh + 64,
                                GSZ * p : GSZ * (p + 1),
                            ],
                        )

                def act_tanh(dst, ps, bias):
                    nc.scalar.activation(dst[:], ps[:], AF.Tanh, bias=bias[:])

                def do_l0(blks):
                    ps0 = qpool.tile([128, 2048], F32, tag="ps")
                    for j in range(SUPER):
                        bi, k = j // 4, j % 4
                        p, qh = j // 2, j % 2
                        nc.tensor.matmul(
                            ps0[64 * qh : 64 * qh + 64, GSZ * p : GSZ * (p + 1)],
                            wt[0][32 * k : 32 * k + 3, :],
                            blks[bi][32 * k : 32 * k + 3, :],
                            tile_position=(32 * k, 64 * qh),
                        )
                        del j
                    h0 = hpool.tile([128, 2048], BF16, tag="hbuf")
                    act_tanh(h0, ps0, bt[0])
                    return h0

                def hidden(rhs_t, li):
                    ps = qpool.tile([128, 2048], F32, tag="ps")
                    mm_layer(ps, rhs_t, wt[li])
                    h = hpool.tile([128, 2048], BF16, tag="hbuf")
                    act_tanh(h, ps, bt[li])
                    return h

                cur_blks, cur_wblks = build_blocks(0)
                cur_h0 = do_l0(cur_blks)
                cur_h1 = hidden(cur_h0, 1)
            for sidx in range(NSUP):
                blks, wblks, h0, h1 = cur_blks, cur_wblks, cur_h0, cur_h1
                r1 = hpool.tile([128, 2048], BF16, tag="hbuf")
                nc.vector.tensor_tensor(r1[:], h1[:], h0[:], OP.add)
                h2 = hidden(r1, 2)
                if sidx + 1 < NSUP:
                    # issue next super's block DMAs early (latency hiding)
                    cur_blks, cur_wblks = build_blocks(sidx + 1)
                h3 = hidden(h2, 3)
                r2 = hpool.tile([128, 2048], BF16, tag="hbuf")
                nc.vector.tensor_tensor(r2[:], h3[:], r1[:], OP.add)
                h4 = hidden(r2, 4)
                if sidx + 1 < NSUP:
                    cur_h0 = do_l0(cur_blks)
                h5 = hidden(h4, 5)
                r3 = hpool.tile([128, 2048], BF16, tag="hbuf")
                nc.vector.tensor_tensor(r3[:], h5[:], r2[:], OP.add)

                # L6 + weighted segmented reduce, 4 groups per psum tile
                for half in range(2):
                    ps6 = qpool.tile([128, 2048], F32, tag="ps")
                    for jj in range(4):
                        j = 4 * half + jj
                        qh = j % 2
                        nc.tensor.matmul(
                            ps6[:, GSZ * jj : GSZ * (jj + 1)],
                            wt[6][64 * qh : 64 * qh + 64, :],
                            r3[
                                64 * qh : 64 * qh + 64,
                                GSZ * (j // 2) : GSZ * (j // 2 + 1),
                            ],
                        )
                    h6 = hpool.tile([128, 2048], BF16, tag="hbuf")
                    act_tanh(h6, ps6, bt[6])

                    prod = hpool.tile([128, 2048], BF16, tag="hbuf")
                    nc.vector.tensor_tensor(prod[:], h6[:], wblks[half][:], OP.mult)
                    col = 16 * sidx + 8 * half
                    nc.vector.tensor_reduce(
                        grt[:, col : col + 8],
                        prod[:].rearrange("p (s x) -> p s x", x=N),
                        mybir.AxisListType.X,
                        OP.add,
                    )
                    if half == 0 and sidx + 1 < NSUP:
                        # fill the tail's PE gap with next super's L1
                        cur_h1 = hidden(cur_h0, 1)

            # ---- tail: normalize GR rows, transpose, store ----
            gsq = spool.tile([128, N], F32, tag="tail")
            nc.vector.tensor_tensor(gsq[:], grt[:], grt[:], OP.mult)
            ps_n = qpool.tile([128, 2048], F32, tag="ps")
            nc.tensor.matmul(ps_n[0:1, 0:N], ones_t[:, 0:1], gsq[:])
            nrm = spool.tile([1, N], F32, tag="tail1")
            nc.scalar.activation(nrm[:], ps_n[0:1, 0:N], AF.Sqrt)
            nc.vector.tensor_single_scalar(nrm[:], nrm[:], EPS, OP.add)
            inv = spool.tile([1, N], F32, tag="tail2")
            nc.vector.reciprocal(inv[:], nrm[:])
            ps_b = qpool.tile([128, 2048], F32, tag="ps")
            nc.tensor.matmul(ps_b[:, 0:N], ones_t[0:1, :], inv[:])
            grn = spool.tile([128, N], F32, tag="tail3")
            nc.vector.tensor_tensor(grn[:], grt[:], ps_b[:, 0:N], OP.mult)

            for th2 in range(2):
                ps_t = qpool.tile([128, 2048], F32, tag="ps")
                nc.tensor.transpose(
                    ps_t[:, 0:128], grn[:, 128 * th2 : 128 * th2 + 128], ident[:]
                )
                ot = spool.tile([128, 128], F32, tag="outt")
                nc.scalar.copy(ot[:], ps_t[:, 0:128])
                nc.sync.dma_start(out_dram[128 * th2 : 128 * th2 + 128, :], ot[:])

    nc.compile()
    return nc


def _get_built():
    global _BUILT
    if _BUILT is None:
        _BUILT = _build()
    return _BUILT


def kernel(**inputs):
    nc = _get_built()
    d = np.ascontiguousarray(np.asarray(inputs["distance_matrices_batch"], np.float32))
    z = np.ascontiguousarray(np.asarray(inputs["atomic_numbers_batch"], np.int32))
    B = d.shape[0]
    in_maps = []
    for c in range(B):
        m = {"d": d[c], "z": z[c]}
        for li in range(7):
            m[f"W{li}"] = np.asarray(inputs[f"W{li}"], np.float32)
            m[f"b{li}"] = np.asarray(inputs[f"b{li}"], np.float32)
        in_maps.append(m)
    res = run_bass_kernel_spmd(nc, in_maps, list(range(B)))
    return np.stack([res.results[c]["out"] for c in range(B)], 0)



# revision 23
# speedup vs baseline: 1.4682x; 1.4682x over previous
"""Trainium2 Bass kernel for DeepRadAEVComputer (B=8, N=256).

Sharding: data-parallel over batch dim B — each of the 8 NeuronCores
processes one molecule (d [256,256], z [256]); weights replicated.

Per-core algorithm (feature-major MLP):
  mask/fc/chem features computed point-major at full 128-partition width,
  then shuffled into feature-major rhs blocks ([3,512] rows per 512-point
  group packed 4 groups per [128,512] SBUF tile at partition bases
  0/32/64/96). The 7-layer tanh MLP runs feature-major with two K=64
  matmuls packed on the PE via disjoint (row,col) tile positions, so the
  per-layer psum is partition-stacked [128, 2048] and every tanh runs at
  full 128-partition ACT width. The cutoff-weighted sum over neighbors j
  is a PE-ones partition-broadcast of w followed by a segmented DVE
  reduce; the final row-normalize uses a PE ones-matvec for the
  partition sum, and the [feat, i] result is PE-transposed for a
  contiguous store. ACT Sin is inaccurate on TRN2, so the cutoff cosine
  is evaluated as an even minimax polynomial on the DVE.
"""

import math
import sys

import numpy as np

sys.path.insert(0, "/opt/trn_rl_repo")

import concourse.bass as bass  # noqa: E402
import concourse.tile as tile  # noqa: E402
from concourse import bacc, masks, mybir  # noqa: E402
from concourse.bass_utils import run_bass_kernel_spmd  # noqa: E402

F32 = mybir.dt.float32
BF16 = mybir.dt.bfloat16
I32 = mybir.dt.int32
AF = mybir.ActivationFunctionType
OP = mybir.AluOpType

N = 256          # atoms per molecule
NPTS = N * N     # 65536 pair-points per core
GSZ = 512        # points per group (2 d-rows)
NGRP = NPTS // GSZ          # 128 groups
SUPER = 8                   # groups per super-tile
NSUP = NGRP // SUPER        # 16 super-tiles
CUTOFF = 5.2
EPS = 1e-7
PI = math.pi

# cos(u)-1 = sum_{k>=1} g_k (u^2)^k minimax on [-pi, pi] (max err 4.6e-10)
GCOS = [
    -0.49999999920407, 0.041666664531272414, -0.0013888869362829136,
    2.4800740359237187e-05, -2.7537667614463017e-07, 2.0625484749459024e-09,
    -9.788307045326216e-12,
]

_BUILT = None


def _build():
    nc = bacc.Bacc("TRN2", target_bir_lowering=False, debug=False)

    d_in = nc.dram_tensor("d", [N, N], F32, kind="ExternalInput")
    z_in = nc.dram_tensor("z", [N], I32, kind="ExternalInput")
    w_dram = []
    b_dram = []
    dims = [(3, 64), (64, 64), (64, 64), (64, 64), (64, 64), (64, 64), (64, 128)]
    for li, (fi, fo) in enumerate(dims):
        w_dram.append(nc.dram_tensor(f"W{li}", [fi, fo], F32, kind="ExternalInput"))
        b_dram.append(nc.dram_tensor(f"b{li}", [fo], F32, kind="ExternalInput"))
    out_dram = nc.dram_tensor("out", [N, 128], F32, kind="ExternalOutput")

    d_flat = d_in[:].rearrange("a b -> (a b)")

    with tile.TileContext(nc) as tc:
        with (
            tc.tile_pool(name="const", bufs=1) as cpool,
            tc.tile_pool(name="prep", bufs=1) as ppool,
            tc.tile_pool(name="scratch", bufs=2) as spool,
            tc.tile_pool(name="rhs", bufs=6) as rpool,
            tc.tile_pool(name="wb", bufs=6) as wpool,
            tc.tile_pool(name="hbuf", bufs=20) as hpool,
            tc.tile_pool(name="psum", bufs=2, space="PSUM") as qpool,
        ):
            # ---- constants ----
            ones_t = cpool.tile([128, 128], F32)
            nc.gpsimd.memset(ones_t[:], 1.0)
            ident = cpool.tile([128, 128], F32)
            masks.make_identity(nc, ident[:])
            eps12 = cpool.tile([128, 1], F32)
            nc.gpsimd.memset(eps12[:], 1e-12)

            # weights: stacked copies so both partition-halves of the PE can
            # run the same layer on two point-groups via tile positions.
            # DMA'd in fp32, then converted to bf16 (PE runs 1 cycle/row on
            # bf16 vs 4 on fp32).
            wt = []
            for li, (fi, fo) in enumerate(dims):
                if li == 0:
                    tf = cpool.tile([128, 64], F32, tag="W0f")
                    for k in range(4):
                        nc.sync.dma_start(tf[32 * k : 32 * k + 3, :], w_dram[0][:])
                    t = cpool.tile([128, 64], BF16, tag="W0")
                    for k in range(4):
                        nc.vector.tensor_copy(
                            t[32 * k : 32 * k + 3, :], tf[32 * k : 32 * k + 3, :]
                        )
                else:
                    tf = cpool.tile([128, fo], F32, tag=f"W{li}f")
                    nc.sync.dma_start(tf[0:64, :], w_dram[li][:])
                    nc.sync.dma_start(tf[64:128, :], w_dram[li][:])
                    t = cpool.tile([128, fo], BF16, tag=f"W{li}")
                    nc.vector.tensor_copy(t[:], tf[:])
                wt.append(t)
            bt = []
            for li, (fi, fo) in enumerate(dims):
                t = cpool.tile([128, 1], F32, tag=f"b{li}")
                bcol = b_dram[li][:].rearrange("(p f) -> p f", f=1)
                if fo == 64:
                    nc.sync.dma_start(t[0:64, :], bcol)
                    nc.sync.dma_start(t[64:128, :], bcol)
                else:
                    nc.sync.dma_start(t[:], bcol)
                bt.append(t)

            # ---- prep: chem + cutoff weights, point-major [i-part, j-free] ----
            zrow_i = ppool.tile([1, N], I32)
            nc.sync.dma_start(zrow_i[:], z_in[:].rearrange("(p f) -> p f", p=1))
            zrow = ppool.tile([1, N], F32)
            nc.vector.tensor_copy(zrow[:], zrow_i[:])

            ps_z = qpool.tile([128, 2048], F32, tag="ps")
            # broadcast z along partitions: ones[1,128].T @ zrow[1,256]
            nc.tensor.matmul(ps_z[:, 0:N], ones_t[0:1, :], zrow[:])

            c1h = []
            c2h = []
            wh = []
            dbh = []
            for hf in range(2):  # i-halves
                zcol_i = ppool.tile([128, 1], I32, tag=f"zcol_i{hf}")
                nc.sync.dma_start(
                    zcol_i[:],
                    z_in[128 * hf : 128 * hf + 128].rearrange("(p f) -> p f", f=1),
                )
                zcol = ppool.tile([128, 1], F32, tag=f"zcol{hf}")
                nc.vector.tensor_copy(zcol[:], zcol_i[:])

                dh = spool.tile([128, N], F32)
                nc.sync.dma_start(dh[:], d_in[128 * hf : 128 * hf + 128, :])
                db = ppool.tile([128, N], BF16, tag=f"db_{hf}")
                nc.scalar.copy(db[:], dh[:])
                dbh.append(db)

                s = spool.tile([128, N], F32)
                nc.vector.tensor_scalar_add(s[:], ps_z[:, 0:N], zcol[:])
                m = spool.tile([128, N], F32)
                nc.vector.tensor_scalar_mul(m[:], ps_z[:, 0:N], zcol[:])
                q = spool.tile([128, N], F32)
                nc.vector.tensor_tensor(q[:], s[:], s[:], OP.mult)
                m2 = spool.tile([128, N], F32)
                nc.vector.tensor_tensor(m2[:], m[:], m[:], OP.mult)
                nc.vector.tensor_tensor(q[:], q[:], m2[:], OP.add)
                sq = spool.tile([128, N], F32)
                nc.scalar.activation(sq[:], q[:], AF.Sqrt, bias=eps12[:])
                fac = spool.tile([128, N], F32)
                nc.vector.reciprocal(fac[:], sq[:])
                c1 = ppool.tile([128, N], BF16, tag=f"c1_{hf}")
                nc.vector.tensor_tensor(c1[:], s[:], fac[:], OP.mult)
                c2 = ppool.tile([128, N], BF16, tag=f"c2_{hf}")
                nc.vector.tensor_tensor(c2[:], m[:], fac[:], OP.mult)

                # w = mask * (0.5*cos(pi*d/CUTOFF)+0.5) via DVE polynomial
                th = spool.tile([128, N], F32)
                nc.vector.tensor_single_scalar(th[:], dh[:], PI / CUTOFF, OP.mult)
                uu = spool.tile([128, N], F32)
                nc.vector.tensor_single_scalar(uu[:], th[:], PI, OP.subtract)
                vv = spool.tile([128, N], F32)
                nc.vector.tensor_tensor(vv[:], uu[:], uu[:], OP.mult)
                pa = spool.tile([128, N], F32)
                pb = spool.tile([128, N], F32)
                nc.vector.tensor_single_scalar(pa[:], vv[:], GCOS[6], OP.mult)
                cur, nxt = pa, pb
                for k in range(5, -1, -1):
                    nc.vector.scalar_tensor_tensor(
                        nxt[:], cur[:], GCOS[k], vv[:], OP.add, OP.mult
                    )
                    cur, nxt = nxt, cur
                ne = spool.tile([128, N], F32)
                nc.vector.tensor_single_scalar(ne[:], dh[:], 0.0, OP.not_equal)
                msk = spool.tile([128, N], F32)
                nc.vector.scalar_tensor_tensor(
                    msk[:], dh[:], CUTOFF, ne[:], OP.is_lt, OP.mult
                )
                fcb = spool.tile([128, N], F32)
                nc.vector.tensor_single_scalar(fcb[:], cur[:], -0.5, OP.mult)
                wv = ppool.tile([128, N], BF16, tag=f"w_{hf}")
                nc.vector.tensor_tensor(wv[:], fcb[:], msk[:], OP.mult)
                c1h.append(c1)
                c2h.append(c2)
                wh.append(wv)

            # GR^T accumulator [feat, i]
            grt = ppool.tile([128, N], F32, tag="grt")

            # ---- MLP super-tiles (8 groups = 4096 points each),
            # software-pipelined: the next super's rhs blocks + L0 matmuls
            # are emitted between this super's mid layers so the PE has
            # independent work while each layer's tanh drains (keeps the
            # PE dense and the HAM clock warm) ----
            def build_blocks(sidx):
                # L0 rhs blocks: 4 groups per [128,512] tile,
                # feature rows at partitions 32k+{0,1,2}
                blks = []
                wblks = []
                for bi in range(2):
                    blk = rpool.tile([128, GSZ], BF16, tag="rhs0")
                    c = 2 * sidx + bi  # block index over 32
                    view = blk[:].rearrange("(k r) f -> k r f", r=32)
                    # block c covers groups 4c..4c+3 => i-rows 8c..8c+7
                    hf = (8 * c) // 128
                    r0 = (8 * c) % 128
                    for feat, src in ((0, dbh[hf]), (1, c1h[hf]), (2, c2h[hf])):
                        nc.gpsimd.dma_start(
                            view[:, feat : feat + 1, :], src[r0 : r0 + 8, :]
                        )
                    blks.append(blk)

                    # w broadcast tile: row 0 = this half-super's 8 i-rows of
                    # w, then log2 partition-doubling copies (DMA engines are
                    # nearly idle; saves a K=1 fp32 matmul per group)
                    wb = wpool.tile([128, 2048], BF16, tag="wb")
                    nc.sync.dma_start(wb[0:1, :], wh[hf][r0 : r0 + 8, :])
                    for k in (1, 2, 4, 8, 16, 32, 64):
                        nc.sync.dma_start(wb[k : 2 * k, :], wb[0:k, :])
                    wblks.append(wb)
                return blks, wblks

            def mm_layer(ps, rhs_t, w_tile):
                # 8 matmuls: group j in super; pair p=j//2, half q=j%2
                for j in range(SUPER):
                    p, qh = j // 2, j % 2
                    nc.tensor.matmul(
                        ps[64 * qh : 64 * qh + 64, GSZ * p : GSZ * (p + 1)],
                        w_tile[64 * qh : 64 * qh + 64, :],
                        rhs_t[
                            64 * qh : 64 * qh + 64,
                            GSZ * p : GSZ * (p + 1),
                        ],
                    )

            def act_tanh(dst, ps, bias):
                nc.scalar.activation(dst[:], ps[:], AF.Tanh, bias=bias[:])

            def do_l0(blks):
                ps0 = qpool.tile([128, 2048], F32, tag="ps")
                for j in range(SUPER):
                    bi, k = j // 4, j % 4
                    p, qh = j // 2, j % 2
                    nc.tensor.matmul(
                        ps0[64 * qh : 64 * qh + 64, GSZ * p : GSZ * (p + 1)],
                        wt[0][32 * k : 32 * k + 3, :],
                        blks[bi][32 * k : 32 * k + 3, :],
                        tile_position=(32 * k, 64 * qh),
                    )
                h0 = hpool.tile([128, 2048], BF16, tag="hbuf")
                act_tanh(h0, ps0, bt[0])
                return h0

            def hidden(rhs_t, li):
                ps = qpool.tile([128, 2048], F32, tag="ps")
                mm_layer(ps, rhs_t, wt[li])
                h = hpool.tile([128, 2048], BF16, tag="hbuf")
                act_tanh(h, ps, bt[li])
                return h

            def resid(a, b):
                r = hpool.tile([128, 2048], BF16, tag="hbuf")
                nc.vector.tensor_tensor(r[:], a[:], b[:], OP.add)
                return r

            def l6_half(r3_t, wblk, half, sidx):
                ps6 = qpool.tile([128, 2048], F32, tag="ps")
                for jj in range(4):
                    j = 4 * half + jj
                    qh = j % 2
                    nc.tensor.matmul(
                        ps6[:, GSZ * jj : GSZ * (jj + 1)],
                        wt[6][64 * qh : 64 * qh + 64, :],
                        r3_t[
                            64 * qh : 64 * qh + 64,
                            GSZ * (j // 2) : GSZ * (j // 2 + 1),
                        ],
                    )
                h6 = hpool.tile([128, 2048], BF16, tag="hbuf")
                act_tanh(h6, ps6, bt[6])
                prod = hpool.tile([128, 2048], BF16, tag="hbuf")
                nc.vector.tensor_tensor(prod[:], h6[:], wblk[:], OP.mult)
                col = 16 * sidx + 8 * half
                nc.vector.tensor_reduce(
                    grt[:, col : col + 8],
                    prod[:].rearrange("p (s x) -> p s x", x=N),
                    mybir.AxisListType.X,
                    OP.add,
                )

            # Two-super lockstep pipeline: supers (2p, 2p+1) advance layer by
            # layer in alternation, so ACT always has the sibling super's psum
            # to drain while the PE refills the slot ACT just freed. Keeps the
            # ACT engine (the per-element tanh floor) near 100% busy and the
            # PE dense enough that the HAM clock stays warm.
            blkA, wbA = build_blocks(0)
            blkB, wbB = build_blocks(1)
            h0A = do_l0(blkA)
            h0B = do_l0(blkB)
            h1A = hidden(h0A, 1)
            h1B = hidden(h0B, 1)
            NPAIR = NSUP // 2
            for pair in range(NPAIR):
                sA, sB = 2 * pair, 2 * pair + 1
                r1A = resid(h1A, h0A)
                r1B = resid(h1B, h0B)
                h2A = hidden(r1A, 2)
                h2B = hidden(r1B, 2)
                if pair + 1 < NPAIR:
                    nblkA, nwbA = build_blocks(2 * pair + 2)
                    nblkB, nwbB = build_blocks(2 * pair + 3)
                h3A = hidden(h2A, 3)
                h3B = hidden(h2B, 3)
                r2A = resid(h3A, r1A)
                r2B = resid(h3B, r1B)
                h4A = hidden(r2A, 4)
                h4B = hidden(r2B, 4)
                h5A = hidden(h4A, 5)
                h5B = hidden(h4B, 5)
                r3A = resid(h5A, r2A)
                r3B = resid(h5B, r2B)
                l6_half(r3A, wbA[0], 0, sA)
                l6_half(r3B, wbB[0], 0, sB)
                if pair + 1 < NPAIR:
                    h0A = do_l0(nblkA)
                l6_half(r3A, wbA[1], 1, sA)
                if pair + 1 < NPAIR:
                    h0B = do_l0(nblkB)
                l6_half(r3B, wbB[1], 1, sB)
                if pair + 1 < NPAIR:
                    h1A = hidden(h0A, 1)
                    h1B = hidden(h0B, 1)
                    blkA, wbA, blkB, wbB = nblkA, nwbA, nblkB, nwbB

            # ---- tail: normalize GR rows, transpose, store ----
            gsq = spool.tile([128, N], F32, tag="tail")
            nc.vector.tensor_tensor(gsq[:], grt[:], grt[:], OP.mult)
            ps_n = qpool.tile([128, 2048], F32, tag="ps")
            nc.tensor.matmul(ps_n[0:1, 0:N], ones_t[:, 0:1], gsq[:])
            nrm = spool.tile([1, N], F32, tag="tail1")
            nc.scalar.activation(nrm[:], ps_n[0:1, 0:N], AF.Sqrt)
            nc.vector.tensor_single_scalar(nrm[:], nrm[:], EPS, OP.add)
            inv = spool.tile([1, N], F32, tag="tail2")
            nc.vector.reciprocal(inv[:], nrm[:])
            ps_b = qpool.tile([128, 2048], F32, tag="ps")
            nc.tensor.matmul(ps_b[:, 0:N], ones_t[0:1, :], inv[:])
            grn = spool.tile([128, N], F32, tag="tail3")
            nc.vector.tensor_tensor(grn[:], grt[:], ps_b[:, 0:N], OP.mult)

            for th2 in range(2):
                ps_t = qpool.tile([128, 2048], F32, tag="ps")
                nc.tensor.transpose(
                    ps_t[:, 0:128], grn[:, 128 * th2 : 128 * th2 + 128], ident[:]
                )
                ot = spool.tile([128, 128], F32, tag="outt")
                nc.scalar.copy(ot[:], ps_t[:, 0:128])
                nc.sync.dma_start(out_dram[128 * th2 : 128 * th2 + 128, :], ot[:])

    nc.compile()
    return nc


def _get_built():
    global _BUILT
    if _BUILT is None:
        _BUILT = _build()
    return _BUILT


def kernel(**inputs):
    nc = _get_built()
    d = np.ascontiguousarray(np.asarray(inputs["distance_matrices_batch"], np.float32))
    z = np.ascontiguousarray(np.asarray(inputs["atomic_numbers_batch"], np.int32))
    B = d.shape[0]
    in_maps = []
    for c in range(B):
        m = {"d": d[c], "z": z[c]}
        for li in range(7):
            m[f"W{li}"] = np.asarray(inputs[f"W{li}"], np.float32)
            m[f"b{li}"] = np.asarray(inputs[f"b{li}"], np.float32)
        in_maps.append(m)
    res = run_bass_kernel_spmd(nc, in_maps, list(range(B)))
    return np.stack([res.results[c]["out"] for c in range(B)], 0)

